# revision 9
# baseline (speedup 1.0000x reference)
"""Trainium2 Bass kernel for nn_FR_PDP_block (dense_cnn).

Strategy: pure data parallelism, B=16 sharded as 2 samples per core over 8
NeuronCores. All parameters replicated. Residual add (+x) on host in fp32.

Per-core pipeline (channels-on-partitions, C=256 = 2 chunks of 128):
  PW1 (PE matmul, f16) -> Act drains fp8 into padded o1p8 [b, 60, 60]
  ALL depthwise taps (xy 25, xz 5, yz 5) on PE as fp8 DoubleRow diagonal
    matmuls: each DR matmul computes 2 taps over a contiguous 480-element
    run (8 full padded rows); wrap lanes discarded at drain. BN scale folded
    into tap weights (host), BN shift + relu fused into drains.
  xy drains on DVE -> accxy; xz/yz drains on Pool (GPSIMD) -> gate bufs;
  gate = sigmoid(relu xz + relu yz) [add DVE, sigmoid Act]; DVE mult.
  PW2 (PE f16, BN2 scale folded) -> Act relu drain with accum_out (SE).
  SE FCs on PE (fp32), relu/sigmoid Act -> per-(chunk,sample) DVE scale.
  store f16; host adds x in fp32.
"""
import sys
from contextlib import ExitStack

import numpy as np

sys.path.insert(0, "/opt/trn_rl_repo")

import concourse.bacc as bacc
import concourse.bass as bass
import concourse.mybir as mybir
import concourse.tile as tile
from concourse import bass2jax

EPS = 1e-5
B, C, H, W = 16, 256, 56, 56
HW = H * W          # 3136
BL = 2              # samples per core
NC_ = 8             # cores
PF = 128            # partitions
NK = C // PF        # 2 channel chunks
NT = 448            # drained pixels per j-tile (8 rows of 56)
NTILES = HW // NT   # 7 per sample
CP = 60             # padded row length (2+56+2)
RP = 61             # padded rows (2+56+2, +1 spill row for shifted runs)
SPP = RP * CP       # 3660 per sample plane
RUN = 8 * CP        # 480: matmul moving run = 8 full padded rows

F16 = mybir.dt.float16
F32 = mybir.dt.float32
F8 = mybir.dt.float8e4
A = mybir.AluOpType
AF = mybir.ActivationFunctionType

# ---- tap orders; tail tap of odd lists must allow a +1-column ghost ----
XY_TAPS = [(dy, dx) for dy in range(5) for dx in range(5) if (dy, dx) != (4, 0)] + [(4, 0)]
XZ_TAPS = [(2, 1), (2, 2), (2, 3), (2, 4), (2, 0)]
YZ_TAPS = [(1, 2), (2, 2), (3, 2), (4, 2), (0, 2)]


def _pairs(taps):
    """[(t0,t1), ...]; odd tail paired with ghost (None -> zero weights)."""
    out = []
    for i in range(0, len(taps) - 1, 2):
        out.append((taps[i], taps[i + 1]))
    if len(taps) % 2:
        out.append((taps[-1], None))
    return out


XY_PAIRS = _pairs(XY_TAPS)   # 13
XZ_PAIRS = _pairs(XZ_TAPS)   # 3
YZ_PAIRS = _pairs(YZ_TAPS)   # 3


def build_module(n_iters: int = 1):
    nc = bacc.Bacc(None, target_bir_lowering=False)

    with tile.TileContext(nc) as tc, ExitStack() as es:
        # ---------------- DRAM I/O ----------------
        x16 = nc.dram_tensor("x16", [NK, PF, BL, HW], F16, kind="ExternalInput").ap()
        w1 = nc.dram_tensor("w1", [NK, NK, PF, PF], F16, kind="ExternalInput").ap()
        w2 = nc.dram_tensor("w2", [NK, NK, PF, PF], F16, kind="ExternalInput").ap()
        dxy = nc.dram_tensor("dxy", [NK, 13, PF, 2 * PF], F8, kind="ExternalInput").ap()
        dxz = nc.dram_tensor("dxz", [NK, 3, PF, 2 * PF], F8, kind="ExternalInput").ap()
        dyz = nc.dram_tensor("dyz", [NK, 3, PF, 2 * PF], F8, kind="ExternalInput").ap()
        tv = nc.dram_tensor("tv", [NK, PF, 4], F32, kind="ExternalInput").ap()
        fc1t = nc.dram_tensor("fc1t", [NK, PF, 16], F32, kind="ExternalInput").ap()
        fc1b = nc.dram_tensor("fc1b", [16, 1], F32, kind="ExternalInput").ap()
        fc2t = nc.dram_tensor("fc2t", [NK, 16, PF], F32, kind="ExternalInput").ap()
        fc2b = nc.dram_tensor("fc2b", [NK, PF, 1], F32, kind="ExternalInput").ap()
        y16 = nc.dram_tensor("y16", [NK, PF, BL, HW], F16, kind="ExternalOutput").ap()

        # ---------------- persistent SBUF ----------------
        const = es.enter_context(tc.tile_pool(name="const", bufs=1))
        xsb = [const.tile([PF, BL, HW], F16, tag=f"xsb{k}", name=f"xsb{k}") for k in range(NK)]
        o1p8 = [const.tile([PF, BL, RP, CP], F8, tag=f"o1p8{k}", name=f"o1p8{k}") for k in range(NK)]
        accxy = [const.tile([PF, BL * HW], F16, tag=f"accxy{k}", name=f"accxy{k}") for k in range(NK)]
        out2 = [const.tile([PF, BL, HW], F16, tag=f"out2{k}", name=f"out2{k}") for k in range(NK)]
        w1sb = const.tile([PF, NK, NK, PF], F16, tag="w1sb")
        w2sb = const.tile([PF, NK, NK, PF], F16, tag="w2sb")
        dxysb = const.tile([PF, NK, 13, 2 * PF], F8, tag="dxysb")
        dxzsb = const.tile([PF, NK, 3, 2 * PF], F8, tag="dxzsb")
        dyzsb = const.tile([PF, NK, 3, 2 * PF], F8, tag="dyzsb")
        tvsb = const.tile([PF, NK, 4], F32, tag="tvsb")
        fc1tsb = const.tile([PF, NK, 16], F32, tag="fc1tsb")
        fc1bsb = const.tile([16, 1], F32, tag="fc1bsb")
        fc2tsb = const.tile([16, NK, PF], F32, tag="fc2tsb")
        fc2bsb = const.tile([PF, NK, 1], F32, tag="fc2bsb")
        sq = [const.tile([PF, 8], F32, tag=f"sq{k}", name=f"sq{k}") for k in range(NK)]
        sqv = [const.tile([PF, BL], F32, tag=f"sqv{k}", name=f"sqv{k}") for k in range(NK)]
        s1sb = const.tile([16, BL], F32, tag="s1sb")
        sesb = [const.tile([PF, BL], F32, tag=f"sesb{k}", name=f"sesb{k}") for k in range(NK)]

        gpool = es.enter_context(tc.tile_pool(name="gpool", bufs=4))

        # ---------------- loads + border zeroing (once) ----------------
        nc.sync.dma_start(out=w1sb[:], in_=w1.rearrange("a b p m -> p a b m"))
        for _b in range(BL):
            for _k in range(NK):
                nc.sync.dma_start(out=xsb[_k][:, _b, :], in_=x16[_k][:, _b, :])
        nc.sync.dma_start(out=w2sb[:], in_=w2.rearrange("a b p m -> p a b m"))
        nc.sync.dma_start(out=dxysb[:], in_=dxy.rearrange("a t p m -> p a t m"))
        nc.sync.dma_start(out=dxzsb[:], in_=dxz.rearrange("a t p m -> p a t m"))
        nc.sync.dma_start(out=dyzsb[:], in_=dyz.rearrange("a t p m -> p a t m"))
        nc.sync.dma_start(out=tvsb[:], in_=tv.rearrange("a p t -> p a t"))
        nc.sync.dma_start(out=fc1tsb[:], in_=fc1t.rearrange("a p t -> p a t"))
        nc.sync.dma_start(out=fc1bsb[:], in_=fc1b)
        nc.sync.dma_start(out=fc2tsb[:], in_=fc2t.rearrange("a p m -> p a m"))
        nc.sync.dma_start(out=fc2bsb[:], in_=fc2b.rearrange("a p o -> p a o"))
        for k in range(NK):
            for b in range(BL):
                nc.vector.memset(o1p8[k][:, b, 0:2, :], 0.0)
                nc.vector.memset(o1p8[k][:, b, 58:61, :], 0.0)
                nc.vector.memset(o1p8[k][:, b, 2:58, 0:2], 0.0)
                nc.vector.memset(o1p8[k][:, b, 2:58, 58:60], 0.0)

        # PSUM: pw pool 2x[128,1024] (PW1 / xy / PW2), zz pool 2x[128,1024]
        pwps = es.enter_context(tc.tile_pool(name="pwps", bufs=2, space="PSUM"))
        zzps = es.enter_context(tc.tile_pool(name="zzps", bufs=2, space="PSUM"))

        # pair list: nt pairs in flat (b, j) order per chunk
        PAIR_NT = [(0, 1), (2, 3), (4, 5), (6, 7), (8, 9), (10, 11), (12, 13)]

        def tap_rhs(k, nt, pair):
            """rhs AP [128, 2, 480] for tap pair over tile nt (b=nt//7, j=nt%7)."""
            b, j = nt // NTILES, nt % NTILES
            (dy0, dx0) = pair[0]
            if pair[1] is None:
                dy1, dx1 = dy0, dx0 + 1  # ghost (zero weights), safe window
            else:
                dy1, dx1 = pair[1]
            off0 = b * SPP + (8 * j + dy0) * CP + dx0
            delta = (dy1 - dy0) * CP + (dx1 - dx0)
            base = o1p8[k][:]
            return bass.AP(
                tensor=base.tensor,
                offset=base.offset + off0,
                ap=[list(base.ap[0]), [delta, 2], [1, RUN]],
            )

        def branch_mms(k, nt0, nt1, pairs, wsb, ps):
            """Accumulate tap pairs for tiles nt0/nt1 into psum halves."""
            n_p = len(pairs)
            for pi, pair in enumerate(pairs):
                for si, nt in enumerate((nt0, nt1)):
                    nc.tensor.matmul(
                        ps[:, si * 512:si * 512 + RUN],
                        lhsT=wsb[:, k, pi].rearrange("p (u v) -> p u v", u=2),
                        rhs=tap_rhs(k, nt, pair),
                        start=(pi == 0), stop=(pi == n_p - 1),
                        perf_mode=mybir.MatmulPerfMode.DoubleRow,
                    )

        def psum_view(ps, si):
            """[128, 8, 56] valid-pixel view of a 480 psum half."""
            return ps[:, si * 512:si * 512 + RUN].rearrange(
                "p (r c) -> p r c", c=CP)[:, :, 0:W]

        def body(_it=0, first=False):
            if not first:
                for b in range(BL):
                    for k in range(NK):
                        nc.sync.dma_start(out=xsb[k][:, b, :], in_=x16[k][:, b, :])

            # ---- PW1 (f16) -> fp8 padded o1p8 ----
            for ko in range(NK):
                for nt0, nt1 in PAIR_NT:
                    ps = pwps.tile([PF, 1024], F32, tag="pw", name=f"pw1_{ko}_{nt0}")
                    for ki in range(NK):
                        for si, nt in enumerate((nt0, nt1)):
                            nc.tensor.matmul(
                                ps[:, si * 512:si * 512 + NT],
                                lhsT=w1sb[:, ki, ko, :],
                                rhs=xsb[ki].rearrange("p b n -> p (b n)")[:, nt * NT:(nt + 1) * NT],
                                start=(ki == 0), stop=(ki == NK - 1),
                            )
                    b0, j0 = nt0 // NTILES, nt0 % NTILES
                    b1, j1 = nt1 // NTILES, nt1 % NTILES
                    if b0 == b1:
                        # contiguous 16 padded rows
                        nc.scalar.copy(
                            out=o1p8[ko][:, b0, 2 + 8 * j0:2 + 8 * j0 + 16, 2:58],
                            in_=ps[:].rearrange("p (s q) -> p s q", q=512)[:, :, 0:NT],
                        )
                    else:
                        for si, (bb, jj) in enumerate(((b0, j0), (b1, j1))):
                            nc.scalar.copy(
                                out=o1p8[ko][:, bb, 2 + 8 * jj:2 + 8 * jj + 8, 2:58],
                                in_=ps[:, si * 512:si * 512 + NT],
                            )

            # ---- branch phase per chunk ----
            for k in range(NK):
                for nt0, nt1 in PAIR_NT:
                    # xy taps (PE DR) -> DVE relu+bias drain -> accxy
                    xps = pwps.tile([PF, 1024], F32, tag="pw", name=f"xy_{k}_{nt0}")
                    branch_mms(k, nt0, nt1, XY_PAIRS, dxysb, xps)
                    for si, nt in enumerate((nt0, nt1)):
                        nc.vector.tensor_scalar(
                            out=accxy[k][:, nt * NT:(nt + 1) * NT].rearrange(
                                "p (r c) -> p r c", c=W),
                            in0=psum_view(xps, si),
                            scalar1=tvsb[:, k, 0:1], scalar2=0.0,
                            op0=A.add, op1=A.max)
                    # xz / yz taps -> Pool relu+bias drains -> gate bufs
                    gxz = gpool.tile([PF, 2, NT], F16, tag="gxz", name=f"gxz{k}_{nt0}")
                    gyz = gpool.tile([PF, 2, NT], F16, tag="gyz", name=f"gyz{k}_{nt0}")
                    zps = zzps.tile([PF, 1024], F32, tag="zz", name=f"xz_{k}_{nt0}")
                    branch_mms(k, nt0, nt1, XZ_PAIRS, dxzsb, zps)
                    for si in range(2):
                        nc.vector.tensor_scalar(
                            out=gxz[:, si].rearrange("p (r c) -> p r c", c=W),
                            in0=psum_view(zps, si),
                            scalar1=tvsb[:, k, 1:2], scalar2=0.0,
                            op0=A.add, op1=A.max)
                    wps = zzps.tile([PF, 1024], F32, tag="zz", name=f"yz_{k}_{nt0}")
                    branch_mms(k, nt0, nt1, YZ_PAIRS, dyzsb, wps)
                    for si in range(2):
                        nc.vector.tensor_scalar(
                            out=gyz[:, si].rearrange("p (r c) -> p r c", c=W),
                            in0=psum_view(wps, si),
                            scalar1=tvsb[:, k, 2:3], scalar2=0.0,
                            op0=A.add, op1=A.max)
                    # gate = sigmoid(gxz + gyz); accxy *= gate
                    gf = gxz[:].rearrange("p s q -> p (s q)")
                    nc.gpsimd.tensor_add(out=gf, in0=gf, in1=gyz[:].rearrange("p s q -> p (s q)"))
                    nc.scalar.activation(out=gf, in_=gf, func=AF.Sigmoid)
                    axf = accxy[k][:, nt0 * NT:(nt0 + 2) * NT]
                    nc.gpsimd.tensor_mul(out=axf, in0=axf, in1=gf)

            # ---- PW2 + BN2+relu + SE squeeze ----
            for b, ko in [(1, 0), (1, 1), (0, 0), (0, 1)]:
                for j0 in range(0, NTILES, 2):
                    js = [j0] if j0 + 1 >= NTILES else [j0, j0 + 1]
                    ps = pwps.tile([PF, 1024], F32, tag="pw", name=f"pw2_{ko}_{b}_{j0}")
                    for ki in range(NK):
                        for si, j in enumerate(js):
                            nt = b * NTILES + j
                            nc.tensor.matmul(
                                ps[:, si * 512:si * 512 + NT],
                                lhsT=w2sb[:, ki, ko, :],
                                rhs=accxy[ki][:, nt * NT:(nt + 1) * NT],
                                start=(ki == 0), stop=(ki == NK - 1),
                            )
                    if len(js) == 2:
                        out_ap = out2[ko][:, b, j0 * NT:(j0 + 2) * NT].rearrange(
                            "p (s q) -> p s q", q=NT)
                        in_ap = ps[:].rearrange("p (s q) -> p s q", q=512)[:, :, 0:NT]
                    else:
                        out_ap = out2[ko][:, b, j0 * NT:(j0 + 1) * NT]
                        in_ap = ps[:, 0:NT]
                    nc.scalar.activation(
                        out=out_ap, in_=in_ap,
                        func=AF.Relu, bias=tvsb[:, ko, 3:4], scale=1.0,
                        accum_out=sq[ko][:, b * 4 + j0 // 2:b * 4 + j0 // 2 + 1],
                    )

            # ---- SE + final scale + store, per sample ----
            for b in (1, 0):
                for k in range(NK):
                    nc.vector.tensor_reduce(
                        out=sqv[k][:, b:b + 1], in_=sq[k][:, b * 4:b * 4 + 4],
                        axis=mybir.AxisListType.X, op=A.add)
                ps1 = pwps.tile([16, 2], F32, tag="pw", name=f"ps1_{b}")
                for k in range(NK):
                    nc.tensor.matmul(ps1[:, 0:1], lhsT=fc1tsb[:, k, :], rhs=sqv[k][:, b:b + 1],
                                     start=(k == 0), stop=(k == NK - 1))
                nc.scalar.activation(out=s1sb[:, b:b + 1], in_=ps1[:, 0:1], func=AF.Relu,
                                     bias=fc1bsb[:], scale=1.0)
                for k in range(NK):
                    ps2 = pwps.tile([PF, 2], F32, tag="pw", name=f"ps2_{b}_{k}")
                    nc.tensor.matmul(ps2[:, 0:1], lhsT=fc2tsb[:, k, :], rhs=s1sb[:, b:b + 1])
                    nc.scalar.activation(out=sesb[k][:, b:b + 1], in_=ps2[:, 0:1],
                                         func=AF.Sigmoid, bias=fc2bsb[:, k, :], scale=1.0)
                for k in range(NK):
                    nc.gpsimd.tensor_scalar(
                        out=out2[k][:, b, :], in0=out2[k][:, b, :],
                        scalar1=sesb[k][:, b:b + 1], scalar2=None, op0=A.mult)
                    nc.sync.dma_start(out=y16[k][:, b, :], in_=out2[k][:, b, :])

        if n_iters == 1:
            body(first=True)
        else:
            with tc.For_i(0, n_iters, 1,
                          hint_engines=(mybir.EngineType.PE,
                                        mybir.EngineType.DVE,
                                        mybir.EngineType.Activation)) as it:
                body(it)

    nc.compile()
    return nc


# ---------------------------------------------------------------------------
# host-side preparation
# ---------------------------------------------------------------------------

def _prep(inputs):
    import ml_dtypes
    f32 = np.float32
    F8NP = ml_dtypes.float8_e4m3
    g = {k: np.asarray(v) for k, v in inputs.items()}

    def fold(p):
        s = (g[f"bn{p}_g"] / np.sqrt(g[f"bn{p}_v"] + EPS)).astype(f32)
        t = (g[f"bn{p}_b"] - g[f"bn{p}_m"] * s).astype(f32)
        return s, t

    s_xy, t_xy = fold("xy")
    s_xz, t_xz = fold("xz")
    s_yz, t_yz = fold("yz")
    s_2, t_2 = fold("2")
    alpha = g["alpha"][0, :, 0, 0].astype(f32)
    beta = g["beta"][0, :, 0, 0].astype(f32)

    kxy = g["xy5_w"][:, 0].astype(f32).copy()
    kxy[:, 1:4, 1:4] += g["xy3_w"][:, 0]
    kxy *= s_xy[:, None, None]
    kxz = g["xz5_w"][:, 0, 0].astype(f32).copy()
    kxz[:, 1:4] += g["xz3_w"][:, 0, 0]
    kxz *= (alpha * s_xz)[:, None]
    kyz = g["yz5_w"][:, 0, :, 0].astype(f32).copy()
    kyz[:, 1:4] += g["yz3_w"][:, 0, :, 0]
    kyz *= (beta * s_yz)[:, None]

    w1t = g["pw1_w"][:, :, 0, 0].T.astype(np.float16)     # [c_in, c_out]
    w2t = (g["pw2_w"][:, :, 0, 0] * s_2[:, None]).T.astype(np.float16)
    w1b = w1t.reshape(NK, PF, NK, PF).transpose(0, 2, 1, 3).copy()  # [ki, ko, 128, 128]
    w2b = w2t.reshape(NK, PF, NK, PF).transpose(0, 2, 1, 3).copy()

    def diag_pairs(kvals, taps, pairs):
        """kvals [C, ...indexed by tap]; -> [NK, n_pairs, PF, 2*PF] fp8 diag."""
        tap_idx = {t: i for i, t in enumerate(taps)}
        out = np.zeros((NK, len(pairs), PF, 2, PF), F8NP)
        kq = kvals.astype(F8NP).astype(f32)
        for k in range(NK):
            for pi, (t0, t1) in enumerate(pairs):
                for kt, t in enumerate((t0, t1)):
                    if t is None:
                        continue
                    v = kq[k * PF:(k + 1) * PF, tap_idx[t]].astype(F8NP)
                    for c in range(PF):
                        out[k, pi, c, kt, c] = v[c]
        return out.reshape(NK, len(pairs), PF, 2 * PF)

    kxy_flat = np.stack([kxy[:, dy, dx] for (dy, dx) in XY_TAPS], axis=1)  # [C,25]
    kxz_flat = np.stack([kxz[:, dx] for (_, dx) in XZ_TAPS], axis=1)       # [C,5]
    kyz_flat = np.stack([kyz[:, dy] for (dy, _) in YZ_TAPS], axis=1)       # [C,5]

    dxy = diag_pairs(kxy_flat, XY_TAPS, XY_PAIRS)
    dxz = diag_pairs(kxz_flat, XZ_TAPS, XZ_PAIRS)
    dyz = diag_pairs(kyz_flat, YZ_TAPS, YZ_PAIRS)

    tv = np.stack([t_xy, alpha * t_xz, beta * t_yz, t_2], axis=1)  # [C,4]

    arrs = {
        "w1": w1b, "w2": w2b, "dxy": dxy, "dxz": dxz, "dyz": dyz,
        "tv": tv.reshape(NK, PF, 4).astype(f32),
        "fc1t": (g["fc1_w"].T / HW).astype(f32).reshape(NK, PF, 16),
        "fc1b": g["fc1_b"].astype(f32).reshape(16, 1),
        "fc2t": g["fc2_w"].T.reshape(16, NK, PF).transpose(1, 0, 2).astype(f32).copy(),
        "fc2b": g["fc2_b"].astype(f32).reshape(NK, PF, 1),
    }
    return arrs


_CACHE = {}


def _get_runner():
    if "runner" in _CACHE:
        return _CACHE["runner"]
    import jax
    import jax.core as jcore
    from jax.sharding import Mesh, PartitionSpec, NamedSharding
    from jax.experimental.shard_map import shard_map

    nc = build_module(n_iters=1)
    bass2jax.install_neuronx_cc_hook()

    in_names, out_names, out_avals, out_shapes = [], [], [], []
    for alloc in nc.m.functions[0].allocations:
        if not isinstance(alloc, mybir.MemoryLocationSet):
            continue
        name = alloc.memorylocations[0].name
        if alloc.kind == "ExternalInput":
            if nc.partition_id_tensor is None or name != nc.partition_id_tensor.name:
                in_names.append(name)
        elif alloc.kind == "ExternalOutput":
            out_names.append(name)
            shape = tuple(alloc.tensor_shape)
            dtype = mybir.dt.np(alloc.dtype)
            out_avals.append(jcore.ShapedArray(shape, dtype))
            out_shapes.append((shape, dtype))
    all_in = list(in_names) + list(out_names)
    if nc.partition_id_tensor is not None:
        all_in.append(nc.partition_id_tensor.name)

    def _body(*args):
        operands = list(args)
        if nc.partition_id_tensor is not None:
            operands.append(bass2jax.partition_id_tensor())
        outs = bass2jax._bass_exec_p.bind(
            *operands, out_avals=tuple(out_avals), in_names=tuple(all_in),
            out_names=tuple(out_names), lowering_input_output_aliases=(),
            sim_require_finite=False, sim_require_nnan=False, nc=nc)
        return tuple(outs)

    devices = jax.devices()[:NC_]
    mesh = Mesh(np.asarray(devices), ("core",))
    nspec = len(in_names) + len(out_names)
    fn = jax.jit(
        shard_map(_body, mesh=mesh,
                  in_specs=(PartitionSpec("core"),) * nspec,
                  out_specs=(PartitionSpec("core"),) * len(out_names),
                  check_rep=False),
        keep_unused=True,
    )
    sharding = NamedSharding(mesh, PartitionSpec("core"))
    _CACHE["runner"] = (fn, in_names, out_names, out_shapes, sharding)
    return _CACHE["runner"]


def kernel(**inputs) -> np.ndarray:
    import jax

    fn, in_names, out_names, out_shapes, sharding = _get_runner()
    x = np.asarray(inputs["x"], np.float32)
    arrs = _prep(inputs)

    percore = {}
    xh = x.astype(np.float16).reshape(NC_, BL, NK, PF, HW).transpose(0, 2, 3, 1, 4)
    percore["x16"] = np.ascontiguousarray(xh.reshape(NC_ * NK, PF, BL, HW))
    for name, a in arrs.items():
        percore[name] = np.concatenate([a] * NC_, axis=0)

    args = [jax.device_put(percore[n], sharding) for n in in_names]
    zeros = [jax.device_put(np.zeros((NC_ * s[0], *s[1:]), d), sharding)
             for (s, d) in out_shapes]
    outs = fn(*args, *zeros)
    y16 = np.asarray(outs[out_names.index("y16")])            # [8*NK, PF, BL, HW]
    y16 = y16.reshape(NC_, NK, PF, BL, HW).transpose(0, 3, 1, 2, 4)
    y = y16.reshape(B, C, H, W).astype(np.float32)
    y += x
    return y
